# revision 27
# baseline (speedup 1.0000x reference)
"""Binarized linear: out = sign(x+eps) @ sign(w+eps).T on 8 trn2 cores.

Sharding: 4x2 grid. Core c=(r,s): rows x[r*2048:(r+1)*2048], rows
w[s*2048:(s+1)*2048]. Each core computes a [2048, 2048] output block; the
host concatenates. No collectives.

Per-core kernel (all arithmetic exact -> rel err 0 vs the f32 reference):
  - binarize x-shard to fp8e4m3 as +/-0.5 (DVE: (x>=0)-0.5), w-shard as +/-1
    (ACT Sign with +1e-20 bias, matching sign(v+1e-20)).
  - transpose to [K-on-partition] layout via PE is_transpose matmuls on
    fp16 PAIRS: two adjacent fp8 k-values ride one 16-bit lane (our fp8
    bytes always form normal fp16 values, so the move is bit-exact), so
    each 128x128 transpose covers 256 fp8 columns -- half the PE transpose
    cycles of fp8-at-a-time. Evictions stay contiguous f16 copies (cheap).
  - the resident operands keep the interleaved pair layout. The matmul
    runs perf_mode=DoubleRowSwInterleave: the stationary x tile is the raw
    interleaved [p, 256] fp8 view (hardware deinterleaves and reads the m
    axis reversed -- the host un-flips each 128-row output block), and the
    moving w is a strided fp8 view (slot stride 1, n stride 2). Both sides
    assign fp8 k value 256*kp + 2p + b to (tile kp, partition p, byte b),
    a consistent permutation of the contraction order. Measured cadence:
    216ns per K=256 x 512-wide pass = the fp8 peak (157 TF/s).
  - DoubleRow passes accumulate K=4096 into fp32 PSUM (products +/-0.5,
    sums exact); eviction scales by 2 -> exact integers, stored fp16
    (integers < 2048 are exact; values here are ~ +/-400) to halve output
    DMA traffic; the host casts back to f32.

Schedule (span ~330us: ~48us DMA-bound startup, x-streaming phase 1,
fully-resident phase 2 at a perfect 216ns/pass, ~11us tail):
  - startup loads w chunks 0+1 (16MB) plus x0/x1; DMA queues process
    concurrently, so the first DR matmul lands ~48us in. Warm matmuls on
    the identity keep the PE clock gate open while loads land.
  - phase 1 streams the remaining x blocks at quarter (0.5MB) granularity
    through a deep staging pool, two output blocks (jc=0,1) per x block
    (6.9us of matmul per 2MB of input keeps it PE-bound), w chunks 2/3
    stream behind in the bus slack.
  - phase 2 (jc=2,3) runs with everything resident: pure DR streams.
  - transpose groups (4 transposes + 1 evict) are pumped one per DR pass;
    out-evictions are emitted one block late so their PE-completion waits
    never head-of-line-block the DVE queue.
Engines: DVE binarizes x + evicts outputs; ACT signs w + evicts x/w
transposes (GPSIMD is ~25x too slow for bulk elementwise and cannot read
PSUM). The Tile scheduler is fed PE timings scaled 2x (build_program
patches TRN2Spec) because the stock cost model prices DR fp8 matmuls at
half their measured hardware cost, which made it emit schedules with
cross-engine waits that bind on hardware.
"""

from collections import deque

import numpy as np

P = 128
GRID_I, GRID_J = 4, 2
N_CORES = 8
FULL_M, FULL_N, FULL_K = 8192, 4096, 4096
M_SH, N_SH = FULL_M // GRID_I, FULL_N // GRID_J  # 2048, 2048

_PROGRAM_CACHE = {}


def build_program(m_sh=M_SH, n_sh=N_SH, k=FULL_K, warmup=64, out_fp16=True):
    """Build (and cache) the per-core Bass program. Same SPMD program on all cores."""
    key = (m_sh, n_sh, k, warmup, out_fp16)
    if key in _PROGRAM_CACHE:
        return _PROGRAM_CACHE[key]

    from contextlib import ExitStack

    import concourse.bass as bass
    import concourse.mybir as mybir
    from concourse import bacc, tile
    from concourse.masks import make_identity

    # The stock cost model prices fp8 DoubleRow matmuls at 0.5 cycles/row;
    # measured hardware cadence is 1.0 cycle/row (216ns per K=256x512 pass).
    # The Tile scheduler interleaves engine queues from these modeled times,
    # so feed it PE timings scaled to match reality (DR dominates PE work;
    # transposes are then modeled conservatively slow, which is benign).
    from concourse import hw_specs as _hw
    _hw.TRN2Spec.PE_CYCLE = 2.0 / 2.4
    _hw.TRN2Spec.PE_CYCLE_PSTATE_MID = 2.0 / 1.2
    _hw.TRN2Spec.PE_CYCLE_PSTATE_LOW = 2.0 / 0.65

    f32 = mybir.dt.float32
    f16 = mybir.dt.float16
    fp8 = mybir.dt.float8e4
    out_dt = f16 if out_fp16 else f32

    KT = k // P          # number of 128-wide k tiles (fp8 layout)
    KT16 = k // (2 * P)  # number of 128-wide uint16-pair tiles
    QH = k // 4          # quarter-row staging width (0.5MB loads pipeline
                         # deeper through the staging pool than 1MB halves)
    IB = m_sh // P       # i blocks (x rows / 128)
    JB = n_sh // P       # j blocks (w rows / 128)
    JBLK = 512           # j chunk width (matmul free dim)
    JC = n_sh // JBLK    # j chunks
    JB_PER_JC = JBLK // P
    TG = 4               # uint16 tiles per transpose-evict group
    assert KT16 % TG == 0

    nc = bacc.Bacc("TRN2", target_bir_lowering=False, debug=False)
    xs = nc.dram_tensor("xs", [m_sh, k], f32, kind="ExternalInput").ap()
    ws = nc.dram_tensor("ws", [n_sh, k], f32, kind="ExternalInput").ap()
    out = nc.dram_tensor("out", [m_sh, n_sh], out_dt, kind="ExternalOutput").ap()

    with tile.TileContext(nc) as tc, ExitStack() as ctx:
        const_pool = ctx.enter_context(tc.tile_pool(name="const", bufs=1))
        stage_x = ctx.enter_context(tc.tile_pool(name="stagex", bufs=6))
        stage_w = ctx.enter_context(tc.tile_pool(name="stagew", bufs=5))
        b8_pool = ctx.enter_context(tc.tile_pool(name="b8", bufs=3))
        xbt_pool = ctx.enter_context(tc.tile_pool(name="xbt", bufs=1))
        wbt_pool = ctx.enter_context(tc.tile_pool(name="wbt", bufs=1))
        out_pool = ctx.enter_context(tc.tile_pool(name="outp", bufs=3))
        psum_t = ctx.enter_context(tc.tile_pool(name="pst", bufs=4, space="PSUM"))
        psum_mm = ctx.enter_context(tc.tile_pool(name="psmm", bufs=3, space="PSUM"))

        ident = const_pool.tile([P, P], fp8, tag="ident")
        make_identity(nc, ident)
        ident16 = const_pool.tile([P, P], f16, tag="ident16")
        nc.vector.tensor_copy(ident16[:], ident[:])
        sign_bias = const_pool.tile([P, 1], f32, tag="sbias")
        nc.any.memset(sign_bias[:], 1e-20)

        # PE warmup: matmuls with no data dependency so the HAM clock-gate
        # opens to 8/8 while the first input DMAs are still in flight.
        psum_wu = ctx.enter_context(tc.tile_pool(name="pswu", bufs=1, space="PSUM"))
        warm_psum = psum_wu.tile([P, P], f32, tag="warm", name="warm") if warmup else None

        def warm(n):
            for _ in range(n):
                nc.tensor.matmul(warm_psum[:], lhsT=ident[:], rhs=ident[:],
                                 start=True, stop=True)

        if warmup:
            warm(warmup)

        # Resident transposed binarized operands as fp16 PAIRS, K on
        # partitions. f16 k-tile kp, partition p, byte b holds fp8 k value
        # 256*kp + 2p + b -- a permutation of the standard tiling, consistent
        # across x and w (contraction order is free). The DR matmul reads the
        # fp8 view with slot stride 1, m/n stride 2.
        xbT = [
            xbt_pool.tile([P, KT16, P], f16, tag=f"xbt{ib}", name=f"xbt{ib}")
            for ib in range(IB)
        ]
        wbT = [
            wbt_pool.tile([P, KT16, JBLK], f16, tag=f"wbt{jc}", name=f"wbt{jc}")
            for jc in range(JC)
        ]

        def bin_x(b8h, stgh):
            # (v >= 0) -> {1,0}; minus 0.5 -> +/-0.5. Matches sign(v+1e-20) up
            # to the measure-zero region (-1e-20, 0) that f32 randn never hits.
            nc.vector.tensor_scalar(
                b8h, stgh, 0.0, 0.5,
                mybir.AluOpType.is_ge, mybir.AluOpType.subtract,
            )

        def bin_w(b8h, stgh):
            nc.scalar.sign(b8h, stgh, bias=sign_bias[:])  # sign(w+1e-20) -> +/-1

        def load_binarize(src_rows, pool, stg_tag, b8_tag, binarize):
            """Load 128 rows x k f32 (four quarter DMAs), binarize to fp8."""
            b8 = b8_pool.tile([P, k], fp8, tag=b8_tag, name=b8_tag)
            for q in range(4):
                stg = pool.tile([P, QH], f32, tag=stg_tag, name=stg_tag)
                nc.sync.dma_start(stg[:], src_rows[:, q * QH:(q + 1) * QH])
                binarize(b8[:, q * QH:(q + 1) * QH], stg[:])
            return b8

        pending = deque()  # transpose-group closures (each ~4 PE transposes)

        def queue_tgroups(b8, dest, dest_col0, evict):
            """Transpose b8 (fp8 [128, k]) into dest[:, :, col0:col0+P] (f16
            pair layout) via fp16-pair PE transposes + one contiguous f16
            eviction per group."""
            for g in range(KT16 // TG):
                def go(g=g, b8=b8, dest=dest, dest_col0=dest_col0, evict=evict):
                    pt = psum_t.tile([P, TG, P], f16, tag="pt", name="pt")
                    for t in range(TG):
                        t16 = g * TG + t
                        in16 = b8[:, 2 * P * t16:2 * P * (t16 + 1)].bitcast(f16)
                        nc.tensor.transpose(pt[:, t, :], in16, ident16[:])
                    evict(dest[:, TG * g:TG * (g + 1), dest_col0:dest_col0 + P],
                          pt[:])
                pending.append(go)

        def pump(n):
            for _ in range(n):
                if not pending:
                    return
                pending.popleft()()

        def prep_x(ib):
            b8 = load_binarize(xs[ib * P:(ib + 1) * P, :], stage_x, "stgx", "xb8",
                               bin_x)
            queue_tgroups(b8, xbT[ib], 0, nc.scalar.copy)  # ACT

        x_state = {"ib": 2, "q": 0, "b8": None}

        def prep_x_half(limit):
            """Advance the x prep stream by two 0.5MB quarters (up to ib
            `limit`); spreads DMA/binarize bursts across the block pair."""
            for _ in range(2):
                ib = x_state["ib"]
                if ib >= limit:
                    return
                q = x_state["q"]
                if q == 0:
                    x_state["b8"] = b8_pool.tile([P, k], fp8, tag="xb8",
                                                 name="xb8")
                b8 = x_state["b8"]
                stg = stage_x.tile([P, QH], f32, tag="stgx", name="stgx")
                nc.sync.dma_start(stg[:], xs[ib * P:(ib + 1) * P,
                                             q * QH:(q + 1) * QH])
                bin_x(b8[:, q * QH:(q + 1) * QH], stg[:])
                if q == 3:
                    queue_tgroups(b8, xbT[ib], 0, nc.scalar.copy)
                    x_state["ib"] += 1
                    x_state["q"] = 0
                else:
                    x_state["q"] = q + 1

        def prep_w(jb):
            jc, sub = divmod(jb, JB_PER_JC)
            b8 = load_binarize(ws[jb * P:(jb + 1) * P, :], stage_w, "stgw", "wb8",
                               bin_w)
            queue_tgroups(b8, wbT[jc], sub * P, nc.vector.tensor_copy)  # DVE

        # Later w chunks stream in at half-block granularity (two 0.5MB
        # quarter loads per step) so their DMA/ACT bursts never displace the
        # critical x-prep chain.
        w_state = {"jb": JB_PER_JC, "q": 0, "b8": None}

        def prep_w_quarter():
            jb = w_state["jb"]
            if jb >= JB:
                return
            q = w_state["q"]
            if q == 0:
                w_state["b8"] = b8_pool.tile([P, k], fp8, tag="wb8", name="wb8")
            b8 = w_state["b8"]
            stg = stage_w.tile([P, QH], f32, tag="stgw", name="stgw")
            nc.sync.dma_start(stg[:], ws[jb * P:(jb + 1) * P, q * QH:(q + 1) * QH])
            bin_w(b8[:, q * QH:(q + 1) * QH], stg[:])
            if q == 3:
                jc, sub = divmod(jb, JB_PER_JC)
                queue_tgroups(b8, wbT[jc], sub * P, nc.vector.tensor_copy)
                w_state["jb"] += 1
                w_state["q"] = 0
            else:
                w_state["q"] = q + 1

        def prep_w_half():
            prep_w_quarter()
            prep_w_quarter()

        def emit_out(ps, ib, jc):
            """Evict + store a finished block. Emitted one iteration late so
            its PE-completion wait is already satisfied when it reaches the
            DVE queue head (no head-of-line blocking of the binarizes)."""
            ob = out_pool.tile([P, JBLK], out_dt, tag="ob", name="ob")
            # products are +/-0.5 (x) * +/-1 (w) = +/-0.5 -> scale by 2
            nc.vector.tensor_scalar_mul(ob[:], ps[:], 2.0)
            nc.sync.dma_start(
                out[ib * P:(ib + 1) * P, jc * JBLK:(jc + 1) * JBLK], ob[:]
            )

        def mm_compute(ib, jc, pump_between=False):
            ps = psum_mm.tile([P, JBLK], f32, tag="ps", name="ps")
            nk = KT16
            for kp in range(nk):
                # Stationary x: raw interleaved fp8 pairs [p, 256] --
                # DoubleRowSwInterleave deinterleaves in hardware and reads
                # the m (column) axis reversed; the host flips each 128-row
                # output block back. Moving w: strided fp8 view (slot stride
                # 1, n stride 2) of the same pair layout.
                lhsT = xbT[ib][:, kp, :].bitcast(fp8)
                rhs = wbT[jc][:, kp, :].bitcast(fp8).rearrange(
                    "p (n two) -> p two n", two=2)
                nc.tensor.matmul(
                    ps[:], lhsT=lhsT, rhs=rhs,
                    start=(kp == 0), stop=(kp == nk - 1),
                    perf_mode=mybir.MatmulPerfMode.DoubleRowSwInterleave,
                )
                if pump_between:
                    pump(1)
            return ps

        # Startup: w chunk 0 plus the first two x blocks (10MB -> first DR
        # matmul possible at ~30us). Pad the PE stream with warmup matmuls so
        # the HAM clock window never sees idle while the startup DMAs land.
        # Issue every startup DMA up front: the staging pools (11 quarter
        # slots in flight) keep the input bus saturated. Then drain the
        # transpose groups with warm-matmul filler so the PE clock gate
        # stays open while the loads land. Startup covers w chunks 0 AND 1:
        # phase 1 pairs (ib,0)+(ib,1) per x block, making the x-streaming
        # phase PE-bound (6.9us of matmul per 2MB x block) with enough bus
        # slack to stream chunks 2/3 behind it.
        # DMA priority order: chunk 0, then x0 (unlocks the first block at
        # ~33us), then chunk 1 (needed by the first jc=1 block), then x1.
        for jb in range(JB_PER_JC):
            prep_w(jb)
        prep_x(0)
        for jb in range(JB_PER_JC, 2 * JB_PER_JC):
            prep_w(jb)
        prep_x(1)
        w_state["jb"] = 2 * JB_PER_JC
        # Drain all transpose groups except x1's (they pump inside phase 1)
        # with warm-matmul filler to hold the PE clock gate open.
        n_drain = len(pending) - 4
        for i in range(n_drain):
            pump(1)
            if warmup:
                warm(2)
            if i % 4 == 0 and warmup:
                warm(4)

        outq = []

        def mm(ib, jc):
            ps = mm_compute(ib, jc, pump_between=True)
            if outq:
                emit_out(*outq.pop(0))
            outq.append((ps, ib, jc))

        # Phase 1: stream x blocks, two output blocks (jc=0,1) per x block;
        # w chunks 2/3 stream behind (two quarters per iteration).
        for ib in range(IB):
            prep_x_half(min(ib + 3, IB))
            prep_w_half()
            mm(ib, 0)
            if ib == 0 and warmup:
                # One-time bridge: chunk 1 is still landing while (0,0) runs.
                warm(48)
            prep_x_half(min(ib + 3, IB))
            mm(ib, 1)
        # Phase 2: everything resident; pure matmul streams.
        for jc in range(2, JC):
            for ib in range(IB):
                prep_w_half()
                mm(ib, jc)
        while outq:
            emit_out(*outq.pop(0))
        while w_state["jb"] < JB:
            prep_w_half()
        pump(len(pending))

    nc.compile()
    _PROGRAM_CACHE[key] = nc
    return nc


def kernel(x, weight):
    x = np.ascontiguousarray(np.asarray(x), dtype=np.float32)
    w = np.ascontiguousarray(np.asarray(weight), dtype=np.float32)
    assert x.shape == (FULL_M, FULL_K) and w.shape == (FULL_N, FULL_K)

    from concourse.bass_utils import run_bass_kernel_spmd

    nc = build_program()
    in_maps = []
    for c in range(N_CORES):
        r, s = divmod(c, GRID_J)
        in_maps.append({
            "xs": x[r * M_SH:(r + 1) * M_SH],
            "ws": w[s * N_SH:(s + 1) * N_SH],
        })
    res = run_bass_kernel_spmd(nc, in_maps, core_ids=list(range(N_CORES))).results
    outp = np.empty((FULL_M, FULL_N), dtype=np.float32)
    for c in range(N_CORES):
        r, s = divmod(c, GRID_J)
        blk = np.asarray(res[c]["out"], dtype=np.float32)
        # SwInterleave reads the stationary m axis reversed: un-flip each
        # 128-row block.
        blk = blk.reshape(M_SH // P, P, N_SH)[:, ::-1, :].reshape(M_SH, N_SH)
        outp[r * M_SH:(r + 1) * M_SH, s * N_SH:(s + 1) * N_SH] = blk
    return outp


# revision 28
# speedup vs baseline: 1.0073x; 1.0073x over previous
"""Binarized linear: out = sign(x+eps) @ sign(w+eps).T on 8 trn2 cores.

Sharding: 4x2 grid. Core c=(r,s): rows x[r*2048:(r+1)*2048], rows
w[s*2048:(s+1)*2048]. Each core computes a [2048, 2048] output block; the
host concatenates. No collectives.

Per-core kernel (all arithmetic exact -> rel err 0 vs the f32 reference):
  - binarize x-shard to fp8e4m3 as +/-0.5 (DVE: (x>=0)-0.5), w-shard as +/-1
    (ACT Sign with +1e-20 bias, matching sign(v+1e-20)).
  - transpose to [K-on-partition] layout via PE is_transpose matmuls on
    fp16 PAIRS: two adjacent fp8 k-values ride one 16-bit lane (our fp8
    bytes always form normal fp16 values, so the move is bit-exact), so
    each 128x128 transpose covers 256 fp8 columns -- half the PE transpose
    cycles of fp8-at-a-time. Evictions stay contiguous f16 copies (cheap).
  - the resident operands keep the interleaved pair layout. The matmul
    runs perf_mode=DoubleRowSwInterleave: the stationary x tile is the raw
    interleaved [p, 256] fp8 view (hardware deinterleaves and reads the m
    axis reversed -- the host un-flips each 128-row output block), and the
    moving w is a strided fp8 view (slot stride 1, n stride 2). Both sides
    assign fp8 k value 256*kp + 2p + b to (tile kp, partition p, byte b),
    a consistent permutation of the contraction order. Measured cadence:
    216ns per K=256 x 512-wide pass = the fp8 peak (157 TF/s).
  - DoubleRow passes accumulate K=4096 into fp32 PSUM (products +/-0.5,
    sums exact); eviction scales by 2 -> exact integers, stored fp16
    (integers < 2048 are exact; values here are ~ +/-400) to halve output
    DMA traffic; the host casts back to f32.

Schedule (span ~330us: ~48us DMA-bound startup, x-streaming phase 1,
fully-resident phase 2 at a perfect 216ns/pass, ~11us tail):
  - startup loads w chunks 0+1 (16MB) plus x0/x1; DMA queues process
    concurrently, so the first DR matmul lands ~48us in. Warm matmuls on
    the identity keep the PE clock gate open while loads land.
  - phase 1 streams the remaining x blocks at quarter (0.5MB) granularity
    through a deep staging pool, two output blocks (jc=0,1) per x block
    (6.9us of matmul per 2MB of input keeps it PE-bound), w chunks 2/3
    stream behind in the bus slack.
  - phase 2 (jc=2,3) runs with everything resident: pure DR streams.
  - transpose groups (4 transposes + 1 evict) are pumped one per DR pass;
    out-evictions are emitted one block late so their PE-completion waits
    never head-of-line-block the DVE queue.
Engines: DVE binarizes x + evicts outputs; ACT signs w + evicts x/w
transposes (GPSIMD is ~25x too slow for bulk elementwise and cannot read
PSUM). The Tile scheduler is fed PE timings scaled 2x (build_program
patches TRN2Spec) because the stock cost model prices DR fp8 matmuls at
half their measured hardware cost, which made it emit schedules with
cross-engine waits that bind on hardware.
"""

from collections import deque

import numpy as np

P = 128
GRID_I, GRID_J = 4, 2
N_CORES = 8
FULL_M, FULL_N, FULL_K = 8192, 4096, 4096
M_SH, N_SH = FULL_M // GRID_I, FULL_N // GRID_J  # 2048, 2048

_PROGRAM_CACHE = {}


def build_program(m_sh=M_SH, n_sh=N_SH, k=FULL_K, warmup=64, out_fp16=True):
    """Build (and cache) the per-core Bass program. Same SPMD program on all cores."""
    key = (m_sh, n_sh, k, warmup, out_fp16)
    if key in _PROGRAM_CACHE:
        return _PROGRAM_CACHE[key]

    from contextlib import ExitStack

    import concourse.bass as bass
    import concourse.mybir as mybir
    from concourse import bacc, tile
    from concourse.masks import make_identity

    # The stock cost model prices fp8 DoubleRow matmuls at 0.5 cycles/row;
    # measured hardware cadence is 1.0 cycle/row (216ns per K=256x512 pass).
    # The Tile scheduler interleaves engine queues from these modeled times,
    # so feed it PE timings scaled to match reality (DR dominates PE work;
    # transposes are then modeled conservatively slow, which is benign).
    from concourse import hw_specs as _hw
    _hw.TRN2Spec.PE_CYCLE = 2.0 / 2.4
    _hw.TRN2Spec.PE_CYCLE_PSTATE_MID = 2.0 / 1.2
    _hw.TRN2Spec.PE_CYCLE_PSTATE_LOW = 2.0 / 0.65

    f32 = mybir.dt.float32
    f16 = mybir.dt.float16
    fp8 = mybir.dt.float8e4
    out_dt = f16 if out_fp16 else f32

    KT = k // P          # number of 128-wide k tiles (fp8 layout)
    KT16 = k // (2 * P)  # number of 128-wide uint16-pair tiles
    QH = k // 4          # quarter-row staging width (0.5MB loads pipeline
                         # deeper through the staging pool than 1MB halves)
    IB = m_sh // P       # i blocks (x rows / 128)
    JB = n_sh // P       # j blocks (w rows / 128)
    JBLK = 512           # j chunk width (matmul free dim)
    JC = n_sh // JBLK    # j chunks
    JB_PER_JC = JBLK // P
    TG = 4               # uint16 tiles per transpose-evict group
    assert KT16 % TG == 0

    nc = bacc.Bacc("TRN2", target_bir_lowering=False, debug=False)
    xs = nc.dram_tensor("xs", [m_sh, k], f32, kind="ExternalInput").ap()
    ws = nc.dram_tensor("ws", [n_sh, k], f32, kind="ExternalInput").ap()
    out = nc.dram_tensor("out", [m_sh, n_sh], out_dt, kind="ExternalOutput").ap()

    with tile.TileContext(nc) as tc, ExitStack() as ctx:
        const_pool = ctx.enter_context(tc.tile_pool(name="const", bufs=1))
        stage_x = ctx.enter_context(tc.tile_pool(name="stagex", bufs=6))
        stage_w = ctx.enter_context(tc.tile_pool(name="stagew", bufs=5))
        b8_pool = ctx.enter_context(tc.tile_pool(name="b8", bufs=3))
        xbt_pool = ctx.enter_context(tc.tile_pool(name="xbt", bufs=1))
        wbt_pool = ctx.enter_context(tc.tile_pool(name="wbt", bufs=1))
        out_pool = ctx.enter_context(tc.tile_pool(name="outp", bufs=3))
        psum_t = ctx.enter_context(tc.tile_pool(name="pst", bufs=4, space="PSUM"))
        psum_mm = ctx.enter_context(tc.tile_pool(name="psmm", bufs=3, space="PSUM"))

        ident = const_pool.tile([P, P], fp8, tag="ident")
        make_identity(nc, ident)
        ident16 = const_pool.tile([P, P], f16, tag="ident16")
        nc.vector.tensor_copy(ident16[:], ident[:])
        sign_bias = const_pool.tile([P, 1], f32, tag="sbias")
        nc.any.memset(sign_bias[:], 1e-20)

        # PE warmup: matmuls with no data dependency so the HAM clock-gate
        # opens to 8/8 while the first input DMAs are still in flight.
        psum_wu = ctx.enter_context(tc.tile_pool(name="pswu", bufs=1, space="PSUM"))
        warm_psum = psum_wu.tile([P, P], f32, tag="warm", name="warm") if warmup else None

        def warm(n):
            for _ in range(n):
                nc.tensor.matmul(warm_psum[:], lhsT=ident[:], rhs=ident[:],
                                 start=True, stop=True)

        if warmup:
            warm(warmup)

        # Resident transposed binarized operands as fp16 PAIRS, K on
        # partitions. f16 k-tile kp, partition p, byte b holds fp8 k value
        # 256*kp + 2p + b -- a permutation of the standard tiling, consistent
        # across x and w (contraction order is free). The DR matmul reads the
        # fp8 view with slot stride 1, m/n stride 2.
        xbT = [
            xbt_pool.tile([P, KT16, P], f16, tag=f"xbt{ib}", name=f"xbt{ib}")
            for ib in range(IB)
        ]
        wbT = [
            wbt_pool.tile([P, KT16, JBLK], f16, tag=f"wbt{jc}", name=f"wbt{jc}")
            for jc in range(JC)
        ]

        def bin_x(b8h, stgh):
            # (v >= 0) -> {1,0}; minus 0.5 -> +/-0.5. Matches sign(v+1e-20) up
            # to the measure-zero region (-1e-20, 0) that f32 randn never hits.
            nc.vector.tensor_scalar(
                b8h, stgh, 0.0, 0.5,
                mybir.AluOpType.is_ge, mybir.AluOpType.subtract,
            )

        def bin_w(b8h, stgh):
            nc.scalar.sign(b8h, stgh, bias=sign_bias[:])  # sign(w+1e-20) -> +/-1

        def load_binarize(src_rows, pool, stg_tag, b8_tag, binarize):
            """Load 128 rows x k f32 (four quarter DMAs), binarize to fp8."""
            b8 = b8_pool.tile([P, k], fp8, tag=b8_tag, name=b8_tag)
            for q in range(4):
                stg = pool.tile([P, QH], f32, tag=stg_tag, name=stg_tag)
                nc.sync.dma_start(stg[:], src_rows[:, q * QH:(q + 1) * QH])
                binarize(b8[:, q * QH:(q + 1) * QH], stg[:])
            return b8

        pending = deque()  # transpose-group closures (each ~4 PE transposes)

        def queue_tgroups(b8, dest, dest_col0, evict):
            """Transpose b8 (fp8 [128, k]) into dest[:, :, col0:col0+P] (f16
            pair layout) via fp16-pair PE transposes + one contiguous f16
            eviction per group."""
            for g in range(KT16 // TG):
                def go(g=g, b8=b8, dest=dest, dest_col0=dest_col0, evict=evict):
                    pt = psum_t.tile([P, TG, P], f16, tag="pt", name="pt")
                    for t in range(TG):
                        t16 = g * TG + t
                        in16 = b8[:, 2 * P * t16:2 * P * (t16 + 1)].bitcast(f16)
                        nc.tensor.transpose(pt[:, t, :], in16, ident16[:])
                    evict(dest[:, TG * g:TG * (g + 1), dest_col0:dest_col0 + P],
                          pt[:])
                pending.append(go)

        def pump(n):
            for _ in range(n):
                if not pending:
                    return
                pending.popleft()()

        def prep_x(ib):
            b8 = load_binarize(xs[ib * P:(ib + 1) * P, :], stage_x, "stgx", "xb8",
                               bin_x)
            queue_tgroups(b8, xbT[ib], 0, nc.scalar.copy)  # ACT

        x_state = {"ib": 2, "q": 0, "b8": None}

        def prep_x_half(limit):
            """Advance the x prep stream by two 0.5MB quarters (up to ib
            `limit`); spreads DMA/binarize bursts across the block pair."""
            for _ in range(2):
                ib = x_state["ib"]
                if ib >= limit:
                    return
                q = x_state["q"]
                if q == 0:
                    x_state["b8"] = b8_pool.tile([P, k], fp8, tag="xb8",
                                                 name="xb8")
                b8 = x_state["b8"]
                stg = stage_x.tile([P, QH], f32, tag="stgx", name="stgx")
                nc.sync.dma_start(stg[:], xs[ib * P:(ib + 1) * P,
                                             q * QH:(q + 1) * QH])
                bin_x(b8[:, q * QH:(q + 1) * QH], stg[:])
                if q == 3:
                    queue_tgroups(b8, xbT[ib], 0, nc.scalar.copy)
                    x_state["ib"] += 1
                    x_state["q"] = 0
                else:
                    x_state["q"] = q + 1

        def prep_w(jb):
            jc, sub = divmod(jb, JB_PER_JC)
            b8 = load_binarize(ws[jb * P:(jb + 1) * P, :], stage_w, "stgw", "wb8",
                               bin_w)
            queue_tgroups(b8, wbT[jc], sub * P, nc.vector.tensor_copy)  # DVE

        # Later w chunks stream in at half-block granularity (two 0.5MB
        # quarter loads per step) so their DMA/ACT bursts never displace the
        # critical x-prep chain.
        w_state = {"jb": JB_PER_JC, "q": 0, "b8": None}

        def prep_w_quarter():
            jb = w_state["jb"]
            if jb >= JB:
                return
            q = w_state["q"]
            if q == 0:
                w_state["b8"] = b8_pool.tile([P, k], fp8, tag="wb8", name="wb8")
            b8 = w_state["b8"]
            stg = stage_w.tile([P, QH], f32, tag="stgw", name="stgw")
            nc.sync.dma_start(stg[:], ws[jb * P:(jb + 1) * P, q * QH:(q + 1) * QH])
            bin_w(b8[:, q * QH:(q + 1) * QH], stg[:])
            if q == 3:
                jc, sub = divmod(jb, JB_PER_JC)
                queue_tgroups(b8, wbT[jc], sub * P, nc.vector.tensor_copy)
                w_state["jb"] += 1
                w_state["q"] = 0
            else:
                w_state["q"] = q + 1

        def prep_w_half():
            prep_w_quarter()
            prep_w_quarter()

        def emit_out(ps, ib, jc):
            """Evict + store a finished block. Emitted one iteration late so
            its PE-completion wait is already satisfied when it reaches the
            DVE queue head (no head-of-line blocking of the binarizes)."""
            ob = out_pool.tile([P, JBLK], out_dt, tag="ob", name="ob")
            # products are +/-0.5 (x) * +/-1 (w) = +/-0.5 -> scale by 2
            nc.vector.tensor_scalar_mul(ob[:], ps[:], 2.0)
            nc.sync.dma_start(
                out[ib * P:(ib + 1) * P, jc * JBLK:(jc + 1) * JBLK], ob[:]
            )

        def mm_compute(ib, jc, pump_between=False):
            ps = psum_mm.tile([P, JBLK], f32, tag="ps", name="ps")
            nk = KT16
            for kp in range(nk):
                # Stationary x: raw interleaved fp8 pairs [p, 256] --
                # DoubleRowSwInterleave deinterleaves in hardware and reads
                # the m (column) axis reversed; the host flips each 128-row
                # output block back. Moving w: strided fp8 view (slot stride
                # 1, n stride 2) of the same pair layout.
                lhsT = xbT[ib][:, kp, :].bitcast(fp8)
                rhs = wbT[jc][:, kp, :].bitcast(fp8).rearrange(
                    "p (n two) -> p two n", two=2)
                nc.tensor.matmul(
                    ps[:], lhsT=lhsT, rhs=rhs,
                    start=(kp == 0), stop=(kp == nk - 1),
                    perf_mode=mybir.MatmulPerfMode.DoubleRowSwInterleave,
                )
                if pump_between:
                    pump(1)
            return ps

        # Startup: w chunk 0 plus the first two x blocks (10MB -> first DR
        # matmul possible at ~30us). Pad the PE stream with warmup matmuls so
        # the HAM clock window never sees idle while the startup DMAs land.
        # Issue every startup DMA up front: the staging pools (11 quarter
        # slots in flight) keep the input bus saturated. Then drain the
        # transpose groups with warm-matmul filler so the PE clock gate
        # stays open while the loads land. Startup covers w chunks 0 AND 1:
        # phase 1 pairs (ib,0)+(ib,1) per x block, making the x-streaming
        # phase PE-bound (6.9us of matmul per 2MB x block) with enough bus
        # slack to stream chunks 2/3 behind it.
        # DMA priority order: chunk 0, then x0 (unlocks the first block at
        # ~33us), then chunk 1 (needed by the first jc=1 block), then x1.
        for jb in range(JB_PER_JC):
            prep_w(jb)
        prep_x(0)
        for jb in range(JB_PER_JC, 2 * JB_PER_JC):
            prep_w(jb)
        prep_x(1)
        w_state["jb"] = 2 * JB_PER_JC
        # Drain all transpose groups except x1's (they pump inside phase 1)
        # with warm-matmul filler to hold the PE clock gate open.
        n_drain = len(pending) - 4
        for i in range(n_drain):
            pump(1)
            if warmup:
                warm(6)
            if i % 4 == 0 and warmup:
                warm(12)

        outq = []

        def mm(ib, jc):
            ps = mm_compute(ib, jc, pump_between=True)
            if outq:
                emit_out(*outq.pop(0))
            outq.append((ps, ib, jc))

        # Phase 1: stream x blocks, two output blocks (jc=0,1) per x block;
        # w chunks 2/3 stream behind (two quarters per iteration).
        for ib in range(IB):
            prep_x_half(min(ib + 3, IB))
            prep_w_half()
            mm(ib, 0)
            if ib == 0 and warmup:
                # One-time bridge: chunk 1 is still landing while (0,0) runs.
                warm(48)
            prep_x_half(min(ib + 3, IB))
            mm(ib, 1)
        # Phase 2: everything resident; pure matmul streams.
        for jc in range(2, JC):
            for ib in range(IB):
                prep_w_half()
                mm(ib, jc)
        while outq:
            emit_out(*outq.pop(0))
        while w_state["jb"] < JB:
            prep_w_half()
        pump(len(pending))

    nc.compile()
    _PROGRAM_CACHE[key] = nc
    return nc


def kernel(x, weight):
    x = np.ascontiguousarray(np.asarray(x), dtype=np.float32)
    w = np.ascontiguousarray(np.asarray(weight), dtype=np.float32)
    assert x.shape == (FULL_M, FULL_K) and w.shape == (FULL_N, FULL_K)

    from concourse.bass_utils import run_bass_kernel_spmd

    nc = build_program()
    in_maps = []
    for c in range(N_CORES):
        r, s = divmod(c, GRID_J)
        in_maps.append({
            "xs": x[r * M_SH:(r + 1) * M_SH],
            "ws": w[s * N_SH:(s + 1) * N_SH],
        })
    res = run_bass_kernel_spmd(nc, in_maps, core_ids=list(range(N_CORES))).results
    outp = np.empty((FULL_M, FULL_N), dtype=np.float32)
    for c in range(N_CORES):
        r, s = divmod(c, GRID_J)
        blk = np.asarray(res[c]["out"], dtype=np.float32)
        # SwInterleave reads the stationary m axis reversed: un-flip each
        # 128-row block.
        blk = blk.reshape(M_SH // P, P, N_SH)[:, ::-1, :].reshape(M_SH, N_SH)
        outp[r * M_SH:(r + 1) * M_SH, s * N_SH:(s + 1) * N_SH] = blk
    return outp
